# revision 34
# baseline (speedup 1.0000x reference)
"""PointGRN (segment_reduce) Trainium2 Bass kernel.

Computation (per segment b, channel c over points feat [N, 64] f32):
    sumsq[b,c]  = sum_{n in seg b} feat[n,c]^2
    r[b,c]      = sqrt(sumsq[b,c])
    rn[b,c]     = r[b,c] / (mean_c r[b,:] + 1e-6)
    out[n,c]    = feat[n,c] * (1 + gamma[c]*rn[b,c]) + beta[c]

Sharding: data-parallel over segments — host reads `offset` and gives each
of the 8 cores one whole segment (padded with zero rows to a 128-row
multiple).  No device-side searchsorted and no collectives needed.

Device kernel (per core), DMA-bound at ~330 GB/s/core:
    pass 1: stream [128 x k*64] f32 tiles (k=32 plus one ragged tail);
            ACT squares into bf16; PE ones-matmul reduces partitions,
            accumulating into 4 PSUM rows.  The first RES tiles stay
            resident in SBUF.
    combine: tiny [1,64] vector math (sqrt + Newton step, mean, scale),
            broadcast scale/beta to [128,128] via a K=1 matmul.
    pass 2: resident tiles are rescaled in place (no reload); the rest are
            re-streamed; y = x*s + beta in place; store.  Loads ride the
            SP HWDGE ring, stores the ACT ring (~332 GB/s combined vs
            ~305 single-ring); +beta alternates DVE/GPSIMD so no engine
            becomes the pass-2 critical path.
"""

import numpy as np

import concourse.bacc as bacc
import concourse.bass as bass
import concourse.mybir as mybir
import concourse.tile as tile
from concourse.bass_utils import run_bass_kernel_spmd

EPS = 1e-06
N_CORES = 8
P = 128          # SBUF partitions
C = 64           # channels
K = 32           # row-groups per partition per full tile
F = K * C        # full-tile free dim (2048 f32 = 8KB/partition)
TILE_ROWS = P * K  # 4096 rows per full tile
MM_N = 512       # matmul moving free-dim chunk
NCHUNK = F // MM_N
RES = 18         # full tiles kept resident in SBUF between the two passes

_AFT = mybir.ActivationFunctionType
_ALU = mybir.AluOpType

_program_cache: dict[tuple, bass.Bass] = {}


def _tile_rows(r_pad):
    """Split r_pad rows into full [128 x K] tiles plus one ragged tail tile."""
    pchunks = r_pad // P
    nt_full = pchunks // K
    k_tail = pchunks % K
    ks = [K] * nt_full + ([k_tail] if k_tail else [])
    return ks


def _build_program(
    r_pad: int,
    repeats: int = 1,
    res: int = RES,
    add_eng: str = "gpsimd",
    bufs_x: int = 4,
) -> bass.Bass:
    """One-core Bass program for a shard of r_pad rows (r_pad % 128 == 0).

    `repeats` re-runs the whole computation body that many times (timing
    only: the wall-clock slope over repeats isolates kernel time from the
    ~80-100ms flat dispatch overhead of this axon environment).
    """
    from contextlib import ExitStack

    ks = _tile_rows(r_pad)
    nt = len(ks)
    res = min(res, sum(1 for k in ks if k == K))
    nc = bacc.Bacc()

    feat = nc.declare_dram_parameter("feat", [r_pad, C], mybir.dt.float32, isOutput=False)
    gamma = nc.declare_dram_parameter("gamma", [1, C], mybir.dt.float32, isOutput=False)
    beta = nc.declare_dram_parameter("beta", [1, C], mybir.dt.float32, isOutput=False)
    out = nc.declare_dram_parameter("out", [r_pad, C], mybir.dt.float32, isOutput=True)

    row0 = [0] * nt
    for t in range(1, nt):
        row0[t] = row0[t - 1] + P * ks[t - 1]

    def feat_view(t):
        r0 = row0[t]
        return feat[r0 : r0 + P * ks[t], :].rearrange("(p k) c -> p (k c)", k=ks[t])

    def out_view(t):
        r0 = row0[t]
        return out[r0 : r0 + P * ks[t], :].rearrange("(p k) c -> p (k c)", k=ks[t])

    with tile.TileContext(nc) as tc, ExitStack() as ctx:
        const = ctx.enter_context(tc.tile_pool(name="const", bufs=1))
        inp = ctx.enter_context(tc.tile_pool(name="inp", bufs=bufs_x))
        resp = ctx.enter_context(tc.tile_pool(name="resp", bufs=1))
        sqp = ctx.enter_context(tc.tile_pool(name="sqp", bufs=2))
        psum = ctx.enter_context(tc.tile_pool(name="psum", bufs=1, space="PSUM"))
        small = ctx.enter_context(tc.tile_pool(name="small", bufs=1))
        adder = getattr(nc, add_eng)

        ones_col = const.tile([P, 1], mybir.dt.bfloat16, name="ones_col", tag="ones_col")
        nc.vector.memset(ones_col, 1.0)
        ones_row = const.tile([1, P], mybir.dt.float32, name="ones_row", tag="ones_row")
        nc.vector.memset(ones_row, 1.0)

        # chunks actually written, and the last tile writing each (stop flag)
        nchunks = (max(ks) * C + MM_N - 1) // MM_N
        last_t_for_chunk = [0] * nchunks
        for t in range(nt):
            for j in range((ks[t] * C + MM_N - 1) // MM_N):
                last_t_for_chunk[j] = t

        for _rep in range(repeats):
            # --- pass 1: sum of squares ----------------------------------
            acc = [
                psum.tile([1, MM_N], mybir.dt.float32, name=f"acc{j}", tag=f"acc{j}")
                for j in range(nchunks)
            ]
            res_tiles = []
            for t in range(nt):
                f_t = ks[t] * C
                if t < res:
                    x = resp.tile([P, F], mybir.dt.float32, name="xr", tag=f"res{t}")
                    res_tiles.append(x)
                    nc.sync.dma_start(out=x[:, :f_t], in_=feat_view(t))
                else:
                    x = inp.tile([P, F], mybir.dt.float32, name="x", tag="x")[:, :f_t]
                    # Pool is idle in pass 1: streamed loads ride SWDGE as a
                    # second descriptor path (SWDGE measured additive, ~346
                    # vs ~328 GB/s on the memcpy probe); a waiting trigger
                    # at Pool's queue head blocks nothing here.
                    nc.gpsimd.dma_start(out=x[:, :f_t], in_=feat_view(t))
                sq = sqp.tile([P, F], mybir.dt.bfloat16, name="sq", tag="sq")
                nc.scalar.activation(sq[:, :f_t], x[:, :f_t], _AFT.Square)
                for j in range((f_t + MM_N - 1) // MM_N):
                    w = min(MM_N, f_t - j * MM_N)
                    nc.tensor.matmul(
                        acc[j][:, :w],
                        lhsT=ones_col[:, :],
                        rhs=sq[:, j * MM_N : j * MM_N + w],
                        start=(t == 0),
                        stop=(t == last_t_for_chunk[j]),
                    )

            # --- combine: [1,64] vector math ------------------------------
            red = small.tile([1, NCHUNK, C], mybir.dt.float32, name="red", tag="red")
            if nchunks < NCHUNK:
                nc.vector.memset(red[:, :, :], 0.0)
            for j in range(nchunks):
                # a chunk may be only partially covered (ragged tail): reduce
                # the written prefix; zero-init handles the rest
                w = min(MM_N, max(ks) * C - j * MM_N)
                kw = w // C
                nc.vector.tensor_reduce(
                    out=red[:, j, :],
                    in_=acc[j][:, : kw * C].rearrange("p (k c) -> p c k", c=C),
                    axis=mybir.AxisListType.X,
                    op=_ALU.add,
                )
            sumsq = small.tile([1, C], mybir.dt.float32, name="sumsq", tag="sumsq")
            nc.vector.tensor_reduce(
                out=sumsq,
                in_=red[:, :, :].rearrange("p k c -> p c k"),
                axis=mybir.AxisListType.X,
                op=_ALU.add,
            )

            # r2 = 2*sqrt(sumsq) via ACT sqrt + one Newton step (ACT sqrt is
            # low precision; Newton with the accurate DVE reciprocal fixes it)
            r0 = small.tile([1, C], mybir.dt.float32, name="r0", tag="r0")
            nc.scalar.activation(r0, sumsq, _AFT.Sqrt)
            rm = small.tile([1, C], mybir.dt.float32, name="rm", tag="rm")
            nc.vector.tensor_scalar_max(rm, r0, 1e-30)
            rinv = small.tile([1, C], mybir.dt.float32, name="rinv", tag="rinv")
            nc.vector.reciprocal(rinv, rm)
            t1 = small.tile([1, C], mybir.dt.float32, name="t1", tag="t1")
            nc.vector.tensor_mul(t1, sumsq, rinv)
            r2 = small.tile([1, C], mybir.dt.float32, name="r2", tag="r2")
            nc.vector.tensor_add(r2, r0, t1)

            # mean + eps:  me = sum(r2)/128 + EPS   (r2 = 2r -> mean = sum/128)
            msum = small.tile([1, 1], mybir.dt.float32, name="msum", tag="msum")
            nc.vector.tensor_reduce(out=msum, in_=r2, axis=mybir.AxisListType.X, op=_ALU.add)
            eps_t = small.tile([1, 1], mybir.dt.float32, name="eps_t", tag="eps_t")
            nc.vector.memset(eps_t, EPS)
            me = small.tile([1, 1], mybir.dt.float32, name="me", tag="me")
            nc.scalar.activation(me, msum, _AFT.Identity, bias=eps_t[:, :], scale=1.0 / (2 * C))
            minv = small.tile([1, 1], mybir.dt.float32, name="minv", tag="minv")
            nc.vector.reciprocal(minv, me)
            mh = small.tile([1, 1], mybir.dt.float32, name="mh", tag="mh")
            nc.vector.tensor_scalar_mul(mh, minv, 0.5)

            # s = 1 + gamma * (r2 * 0.5 * minv); pack [s | beta] in one row
            g_row = small.tile([1, C], mybir.dt.float32, name="g_row", tag="g_row")
            nc.sync.dma_start(out=g_row, in_=gamma[:])
            t2 = small.tile([1, C], mybir.dt.float32, name="t2", tag="t2")
            nc.vector.tensor_mul(t2, r2, g_row)
            sb_cat = small.tile([1, 2 * C], mybir.dt.float32, name="sb_cat", tag="sb_cat")
            nc.vector.tensor_scalar(
                sb_cat[:, 0:C], t2, scalar1=mh[:, :], scalar2=1.0, op0=_ALU.mult, op1=_ALU.add
            )
            nc.sync.dma_start(out=sb_cat[:, C : 2 * C], in_=beta[:])

            # broadcast [1,128] -> [128,128]: cols 0-63 = s, 64-127 = beta
            bc_ps = psum.tile([P, 2 * C], mybir.dt.float32, name="bc_ps", tag="bc_ps")
            nc.tensor.matmul(bc_ps[:, :], lhsT=ones_row[:, :], rhs=sb_cat[:, :], start=True, stop=True)
            sb_bc = small.tile([P, 2 * C], mybir.dt.float32, name="sb_bc", tag="sb_bc")
            nc.scalar.copy(sb_bc, bc_ps)
            s_bc = sb_bc[:, 0:C]
            b_bc = sb_bc[:, C : 2 * C]

            def bcast_ap(col_slice, kk):
                return bass.AP(
                    tensor=col_slice.tensor,
                    offset=col_slice.offset,
                    ap=[col_slice.ap[0], [0, kk], col_slice.ap[1]],
                )

            def rescale(x, t):
                kk = ks[t]
                x3 = x[:, : kk * C].rearrange("p (k c) -> p k c", c=C)
                nc.vector.tensor_tensor(x3, x3, bcast_ap(s_bc, kk), _ALU.mult)
                # alternate the +beta between Pool and DVE so neither engine
                # becomes the pass-2 critical path
                eng = adder if t % 2 == 0 else nc.vector
                eng.tensor_tensor(x3, x3, bcast_ap(b_bc, kk), _ALU.add)

            # --- pass 2: y = x*s + beta (in place) ------------------------
            # resident tiles first (no loads; fills the combine bubble while
            # streamed loads prefetch), stored from SBUF directly
            for t in range(res):
                x = res_tiles[t]
                rescale(x, t)
                nc.scalar.dma_start(out=out_view(t), in_=x[:, : ks[t] * C])
            for t in range(res, nt):
                f_t = ks[t] * C
                x = inp.tile([P, F], mybir.dt.float32, name="x", tag="x")[:, :f_t]
                nc.sync.dma_start(out=x, in_=feat_view(t))
                rescale(x, t)
                nc.scalar.dma_start(out=out_view(t), in_=x)

    nc.finalize()
    return nc


def kernel(feat: np.ndarray, offset: np.ndarray, gamma: np.ndarray, beta: np.ndarray) -> np.ndarray:
    feat = np.ascontiguousarray(np.asarray(feat, dtype=np.float32))
    offset = np.asarray(offset)
    gamma = np.ascontiguousarray(np.asarray(gamma, dtype=np.float32)).reshape(1, C)
    beta = np.ascontiguousarray(np.asarray(beta, dtype=np.float32)).reshape(1, C)

    n = feat.shape[0]
    b = offset.shape[0]
    assert b <= N_CORES, f"need <= {N_CORES} segments, got {b}"

    ends = offset.astype(np.int64)
    starts = np.concatenate([[0], ends[:-1]])
    seg_rows = (ends - starts).astype(np.int64)

    r_max = int(seg_rows.max()) if b else P
    r_pad = max(P, ((r_max + P - 1) // P) * P)

    key = (r_pad,)
    nc = _program_cache.get(key)
    if nc is None:
        nc = _build_program(r_pad)
        _program_cache[key] = nc

    in_maps = []
    for i in range(N_CORES):
        shard = np.zeros((r_pad, C), dtype=np.float32)
        if i < b and seg_rows[i] > 0:
            shard[: seg_rows[i]] = feat[starts[i] : ends[i]]
        in_maps.append({"feat": shard, "gamma": gamma, "beta": beta})

    results = run_bass_kernel_spmd(nc, in_maps, core_ids=list(range(N_CORES))).results

    out_full = np.empty((n, C), dtype=np.float32)
    for i in range(b):
        if seg_rows[i] > 0:
            out_full[starts[i] : ends[i]] = results[i]["out"][: seg_rows[i]]

    # Rows past offset[-1] (possible with general sorted offsets): the
    # reference's searchsorted yields index b there, which jax clamps to
    # b-1 on gather — those rows are scaled by the last segment's rn but
    # excluded from its sumsq.  Replicate on host.
    tail0 = int(ends[-1]) if b else 0
    if tail0 < n:
        last0, last1 = int(starts[-1]), int(ends[-1])
        sumsq = (feat[last0:last1].astype(np.float64) ** 2).sum(axis=0)
        r = np.sqrt(sumsq)
        rn = (r / (r.mean() + EPS)).astype(np.float32)
        ft = feat[tail0:]
        out_full[tail0:] = ft + gamma * (ft * rn[None, :]) + beta
    return out_full


# revision 35
# speedup vs baseline: 1.0474x; 1.0474x over previous
"""PointGRN (segment_reduce) Trainium2 Bass kernel.

Computation (per segment b, channel c over points feat [N, 64] f32):
    sumsq[b,c]  = sum_{n in seg b} feat[n,c]^2
    r[b,c]      = sqrt(sumsq[b,c])
    rn[b,c]     = r[b,c] / (mean_c r[b,:] + 1e-6)
    out[n,c]    = feat[n,c] * (1 + gamma[c]*rn[b,c]) + beta[c]

Sharding: data-parallel over segments — host reads `offset` and gives each
of the 8 cores one whole segment (padded with zero rows to a 128-row
multiple).  No device-side searchsorted and no collectives needed.

Device kernel (per core), DMA-bound at ~330 GB/s/core:
    pass 1: stream [128 x k*64] f32 tiles (k=32 plus one ragged tail);
            ACT squares into bf16; PE ones-matmul reduces partitions,
            accumulating into 4 PSUM rows.  The first RES tiles stay
            resident in SBUF.
    combine: tiny [1,64] vector math (sqrt + Newton step, mean, scale),
            broadcast scale/beta to [128,128] via a K=1 matmul.
    pass 2: resident tiles are rescaled in place (no reload); the rest are
            re-streamed; y = x*s + beta in place; store.  Loads ride the
            SP HWDGE ring, stores the ACT ring (~332 GB/s combined vs
            ~305 single-ring); +beta alternates DVE/GPSIMD so no engine
            becomes the pass-2 critical path.
"""

import numpy as np

import concourse.bacc as bacc
import concourse.bass as bass
import concourse.mybir as mybir
import concourse.tile as tile
from concourse.bass_utils import run_bass_kernel_spmd

EPS = 1e-06
N_CORES = 8
P = 128          # SBUF partitions
C = 64           # channels
K = 32           # row-groups per partition per full tile
F = K * C        # full-tile free dim (2048 f32 = 8KB/partition)
TILE_ROWS = P * K  # 4096 rows per full tile
MM_N = 512       # matmul moving free-dim chunk
NCHUNK = F // MM_N
RES = 20         # full tiles kept resident in SBUF between the two passes

_AFT = mybir.ActivationFunctionType
_ALU = mybir.AluOpType

_program_cache: dict[tuple, bass.Bass] = {}


def _tile_rows(r_pad):
    """Split r_pad rows into full [128 x K] tiles plus one ragged tail tile."""
    pchunks = r_pad // P
    nt_full = pchunks // K
    k_tail = pchunks % K
    ks = [K] * nt_full + ([k_tail] if k_tail else [])
    return ks


def _build_program(
    r_pad: int,
    repeats: int = 1,
    res: int = RES,
    add_eng: str = "gpsimd",
    bufs_x: int = 4,
) -> bass.Bass:
    """One-core Bass program for a shard of r_pad rows (r_pad % 128 == 0).

    `repeats` re-runs the whole computation body that many times (timing
    only: the wall-clock slope over repeats isolates kernel time from the
    ~80-100ms flat dispatch overhead of this axon environment).
    """
    from contextlib import ExitStack

    ks = _tile_rows(r_pad)
    nt = len(ks)
    res = min(res, sum(1 for k in ks if k == K))
    nc = bacc.Bacc()

    feat = nc.declare_dram_parameter("feat", [r_pad, C], mybir.dt.float32, isOutput=False)
    gamma = nc.declare_dram_parameter("gamma", [1, C], mybir.dt.float32, isOutput=False)
    beta = nc.declare_dram_parameter("beta", [1, C], mybir.dt.float32, isOutput=False)
    out = nc.declare_dram_parameter("out", [r_pad, C], mybir.dt.float32, isOutput=True)

    row0 = [0] * nt
    for t in range(1, nt):
        row0[t] = row0[t - 1] + P * ks[t - 1]

    def feat_view(t):
        r0 = row0[t]
        return feat[r0 : r0 + P * ks[t], :].rearrange("(p k) c -> p (k c)", k=ks[t])

    def out_view(t):
        r0 = row0[t]
        return out[r0 : r0 + P * ks[t], :].rearrange("(p k) c -> p (k c)", k=ks[t])

    with tile.TileContext(nc) as tc, ExitStack() as ctx:
        const = ctx.enter_context(tc.tile_pool(name="const", bufs=1))
        inp = ctx.enter_context(tc.tile_pool(name="inp", bufs=bufs_x))
        resp = ctx.enter_context(tc.tile_pool(name="resp", bufs=1))
        sqp = ctx.enter_context(tc.tile_pool(name="sqp", bufs=2))
        psum = ctx.enter_context(tc.tile_pool(name="psum", bufs=1, space="PSUM"))
        small = ctx.enter_context(tc.tile_pool(name="small", bufs=1))
        adder = getattr(nc, add_eng)

        ones_col = const.tile([P, 1], mybir.dt.bfloat16, name="ones_col", tag="ones_col")
        nc.vector.memset(ones_col, 1.0)
        ones_row = const.tile([1, P], mybir.dt.float32, name="ones_row", tag="ones_row")
        nc.vector.memset(ones_row, 1.0)

        # chunks actually written, and the last tile writing each (stop flag)
        nchunks = (max(ks) * C + MM_N - 1) // MM_N
        last_t_for_chunk = [0] * nchunks
        for t in range(nt):
            for j in range((ks[t] * C + MM_N - 1) // MM_N):
                last_t_for_chunk[j] = t

        for _rep in range(repeats):
            # --- pass 1: sum of squares ----------------------------------
            acc = [
                psum.tile([1, MM_N], mybir.dt.float32, name=f"acc{j}", tag=f"acc{j}")
                for j in range(nchunks)
            ]
            res_tiles = []
            for t in range(nt):
                f_t = ks[t] * C
                if t < res:
                    x = resp.tile([P, F], mybir.dt.float32, name="xr", tag=f"res{t}")
                    res_tiles.append(x)
                    nc.sync.dma_start(out=x[:, :f_t], in_=feat_view(t))
                else:
                    x = inp.tile([P, F], mybir.dt.float32, name="x", tag="x")[:, :f_t]
                    # Pool is idle in pass 1: streamed loads ride SWDGE as a
                    # second descriptor path (SWDGE measured additive, ~346
                    # vs ~328 GB/s on the memcpy probe); a waiting trigger
                    # at Pool's queue head blocks nothing here.
                    nc.gpsimd.dma_start(out=x[:, :f_t], in_=feat_view(t))
                sq = sqp.tile([P, F], mybir.dt.bfloat16, name="sq", tag="sq")
                nc.scalar.activation(sq[:, :f_t], x[:, :f_t], _AFT.Square)
                for j in range((f_t + MM_N - 1) // MM_N):
                    w = min(MM_N, f_t - j * MM_N)
                    nc.tensor.matmul(
                        acc[j][:, :w],
                        lhsT=ones_col[:, :],
                        rhs=sq[:, j * MM_N : j * MM_N + w],
                        start=(t == 0),
                        stop=(t == last_t_for_chunk[j]),
                    )

            # --- combine: [1,64] vector math ------------------------------
            red = small.tile([1, NCHUNK, C], mybir.dt.float32, name="red", tag="red")
            if nchunks < NCHUNK:
                nc.vector.memset(red[:, :, :], 0.0)
            for j in range(nchunks):
                # a chunk may be only partially covered (ragged tail): reduce
                # the written prefix; zero-init handles the rest
                w = min(MM_N, max(ks) * C - j * MM_N)
                kw = w // C
                nc.vector.tensor_reduce(
                    out=red[:, j, :],
                    in_=acc[j][:, : kw * C].rearrange("p (k c) -> p c k", c=C),
                    axis=mybir.AxisListType.X,
                    op=_ALU.add,
                )
            sumsq = small.tile([1, C], mybir.dt.float32, name="sumsq", tag="sumsq")
            nc.vector.tensor_reduce(
                out=sumsq,
                in_=red[:, :, :].rearrange("p k c -> p c k"),
                axis=mybir.AxisListType.X,
                op=_ALU.add,
            )

            # r2 = 2*sqrt(sumsq) via ACT sqrt + one Newton step (ACT sqrt is
            # low precision; Newton with the accurate DVE reciprocal fixes it)
            r0 = small.tile([1, C], mybir.dt.float32, name="r0", tag="r0")
            nc.scalar.activation(r0, sumsq, _AFT.Sqrt)
            rm = small.tile([1, C], mybir.dt.float32, name="rm", tag="rm")
            nc.vector.tensor_scalar_max(rm, r0, 1e-30)
            rinv = small.tile([1, C], mybir.dt.float32, name="rinv", tag="rinv")
            nc.vector.reciprocal(rinv, rm)
            t1 = small.tile([1, C], mybir.dt.float32, name="t1", tag="t1")
            nc.vector.tensor_mul(t1, sumsq, rinv)
            r2 = small.tile([1, C], mybir.dt.float32, name="r2", tag="r2")
            nc.vector.tensor_add(r2, r0, t1)

            # mean + eps:  me = sum(r2)/128 + EPS   (r2 = 2r -> mean = sum/128)
            msum = small.tile([1, 1], mybir.dt.float32, name="msum", tag="msum")
            nc.vector.tensor_reduce(out=msum, in_=r2, axis=mybir.AxisListType.X, op=_ALU.add)
            eps_t = small.tile([1, 1], mybir.dt.float32, name="eps_t", tag="eps_t")
            nc.vector.memset(eps_t, EPS)
            me = small.tile([1, 1], mybir.dt.float32, name="me", tag="me")
            nc.scalar.activation(me, msum, _AFT.Identity, bias=eps_t[:, :], scale=1.0 / (2 * C))
            minv = small.tile([1, 1], mybir.dt.float32, name="minv", tag="minv")
            nc.vector.reciprocal(minv, me)
            mh = small.tile([1, 1], mybir.dt.float32, name="mh", tag="mh")
            nc.vector.tensor_scalar_mul(mh, minv, 0.5)

            # s = 1 + gamma * (r2 * 0.5 * minv); pack [s | beta] in one row
            g_row = small.tile([1, C], mybir.dt.float32, name="g_row", tag="g_row")
            nc.sync.dma_start(out=g_row, in_=gamma[:])
            t2 = small.tile([1, C], mybir.dt.float32, name="t2", tag="t2")
            nc.vector.tensor_mul(t2, r2, g_row)
            sb_cat = small.tile([1, 2 * C], mybir.dt.float32, name="sb_cat", tag="sb_cat")
            nc.vector.tensor_scalar(
                sb_cat[:, 0:C], t2, scalar1=mh[:, :], scalar2=1.0, op0=_ALU.mult, op1=_ALU.add
            )
            nc.sync.dma_start(out=sb_cat[:, C : 2 * C], in_=beta[:])

            # broadcast [1,128] -> [128,128]: cols 0-63 = s, 64-127 = beta
            bc_ps = psum.tile([P, 2 * C], mybir.dt.float32, name="bc_ps", tag="bc_ps")
            nc.tensor.matmul(bc_ps[:, :], lhsT=ones_row[:, :], rhs=sb_cat[:, :], start=True, stop=True)
            sb_bc = small.tile([P, 2 * C], mybir.dt.float32, name="sb_bc", tag="sb_bc")
            nc.scalar.copy(sb_bc, bc_ps)
            s_bc = sb_bc[:, 0:C]
            b_bc = sb_bc[:, C : 2 * C]

            def bcast_ap(col_slice, kk):
                return bass.AP(
                    tensor=col_slice.tensor,
                    offset=col_slice.offset,
                    ap=[col_slice.ap[0], [0, kk], col_slice.ap[1]],
                )

            def rescale(x, t):
                kk = ks[t]
                x3 = x[:, : kk * C].rearrange("p (k c) -> p k c", c=C)
                nc.vector.tensor_tensor(x3, x3, bcast_ap(s_bc, kk), _ALU.mult)
                # alternate the +beta between Pool and DVE so neither engine
                # becomes the pass-2 critical path
                eng = adder if t % 2 == 0 else nc.vector
                eng.tensor_tensor(x3, x3, bcast_ap(b_bc, kk), _ALU.add)

            # --- pass 2: y = x*s + beta (in place) ------------------------
            # resident tiles first (no loads; fills the combine bubble while
            # streamed loads prefetch), stored from SBUF directly
            for t in range(res):
                x = res_tiles[t]
                rescale(x, t)
                nc.scalar.dma_start(out=out_view(t), in_=x[:, : ks[t] * C])
            for t in range(res, nt):
                f_t = ks[t] * C
                x = inp.tile([P, F], mybir.dt.float32, name="x", tag="x")[:, :f_t]
                nc.sync.dma_start(out=x, in_=feat_view(t))
                rescale(x, t)
                nc.scalar.dma_start(out=out_view(t), in_=x)

    nc.finalize()
    return nc


def kernel(feat: np.ndarray, offset: np.ndarray, gamma: np.ndarray, beta: np.ndarray) -> np.ndarray:
    feat = np.ascontiguousarray(np.asarray(feat, dtype=np.float32))
    offset = np.asarray(offset)
    gamma = np.ascontiguousarray(np.asarray(gamma, dtype=np.float32)).reshape(1, C)
    beta = np.ascontiguousarray(np.asarray(beta, dtype=np.float32)).reshape(1, C)

    n = feat.shape[0]
    b = offset.shape[0]
    assert b <= N_CORES, f"need <= {N_CORES} segments, got {b}"

    ends = offset.astype(np.int64)
    starts = np.concatenate([[0], ends[:-1]])
    seg_rows = (ends - starts).astype(np.int64)

    r_max = int(seg_rows.max()) if b else P
    r_pad = max(P, ((r_max + P - 1) // P) * P)

    key = (r_pad,)
    nc = _program_cache.get(key)
    if nc is None:
        nc = _build_program(r_pad)
        _program_cache[key] = nc

    in_maps = []
    for i in range(N_CORES):
        shard = np.zeros((r_pad, C), dtype=np.float32)
        if i < b and seg_rows[i] > 0:
            shard[: seg_rows[i]] = feat[starts[i] : ends[i]]
        in_maps.append({"feat": shard, "gamma": gamma, "beta": beta})

    results = run_bass_kernel_spmd(nc, in_maps, core_ids=list(range(N_CORES))).results

    out_full = np.empty((n, C), dtype=np.float32)
    for i in range(b):
        if seg_rows[i] > 0:
            out_full[starts[i] : ends[i]] = results[i]["out"][: seg_rows[i]]

    # Rows past offset[-1] (possible with general sorted offsets): the
    # reference's searchsorted yields index b there, which jax clamps to
    # b-1 on gather — those rows are scaled by the last segment's rn but
    # excluded from its sumsq.  Replicate on host.
    tail0 = int(ends[-1]) if b else 0
    if tail0 < n:
        last0, last1 = int(starts[-1]), int(ends[-1])
        sumsq = (feat[last0:last1].astype(np.float64) ** 2).sum(axis=0)
        r = np.sqrt(sumsq)
        rn = (r / (r.mean() + EPS)).astype(np.float32)
        ft = feat[tail0:]
        out_full[tail0:] = ft + gamma * (ft * rn[None, :]) + beta
    return out_full
